# revision 5
# baseline (speedup 1.0000x reference)
"""v5: F-table kernel for ChannelwiseSpatialMHSA (GX=64).

The attention collapses to out[s,:] = sum_h w(c_h*x_s)*mu_h = F(x_s) where
w(a) = sum_t softmax_t(a*x_t)*x_t and F: R -> R^64 per sequence. Tabulate F
on a GX=64 uniform x-grid (256 tilts = 2 [128,1024] Exp+STT passes per seq,
w evaluated EXACTLY at the tilts), then linear-interpolate F at the 1024
query positions x_s with a triangle-kernel weight matrix, folded into the
head/output projection + channel merge as K=128 matmul accumulations
(2 seqs stacked per matmul group). Triangle weights are built as
B' = min(|x*k1 + bias_g| - 1, 0) = -relu(1-|.|) (one Abs ACT + one vector
tensor_scalar); mu is negated on host so the signs cancel in the matmul.
Measured rel err ~2.2e-3 (budget 2e-2).

All weight folding (c_h, U = heads(v)@o^T, merge scaling, exp scale/bias)
is host-side; device inputs are xs + 3 small constant tables.
"""

import numpy as np

B, HH, WW, C = 2, 32, 32, 32
S = 1024
D = 64
NH = 4
DH = 16
NCORES = 8
NSEQ = 8
GX = 64  # x-grid points per sequence; 2 tilt tiles (4h x 32g) per seq
NST = NSEQ // 2  # B stacks of 2 sequences (K = 2*GX = 128)

_CACHE = {}


def _build_nc():
    import concourse.bacc as bacc
    import concourse.bass as bass
    import concourse.tile as tile
    from concourse import mybir

    f32 = mybir.dt.float32
    Alu = mybir.AluOpType
    Act = mybir.ActivationFunctionType

    nc = bacc.Bacc()

    xs = nc.dram_tensor("xs", [NSEQ, S], f32, kind="ExternalInput")
    # per (seq, tilt-tile): exp scale col, bias col
    scb = nc.dram_tensor("scb", [128, 4 * NSEQ], f32, kind="ExternalInput")
    # per stack: B scale col (k1), bias col (31.5 - g)
    scbB = nc.dram_tensor("scbB", [128, 2 * NST], f32, kind="ExternalInput")
    mu = nc.dram_tensor("mu", [NH, NSEQ * D], f32, kind="ExternalInput")
    outp = nc.dram_tensor("outp", [S, D], f32, kind="ExternalOutput")

    with tile.TileContext(nc) as tc:
        with (
            tc.tile_pool(name="consts", bufs=1) as consts,
            tc.tile_pool(name="xall", bufs=1) as xallp,
            tc.tile_pool(name="et", bufs=3) as etp,
            tc.tile_pool(name="scr", bufs=3) as scrp,
            tc.tile_pool(name="x4", bufs=2) as x4p,
            tc.tile_pool(name="babs", bufs=2) as babsp,
            tc.tile_pool(name="bst", bufs=2) as bstp,
            tc.tile_pool(name="fstk", bufs=2) as fstkp,
            tc.tile_pool(name="small", bufs=12) as smallp,
            tc.tile_pool(name="fps", bufs=2, space="PSUM") as fpsp,
            tc.tile_pool(name="accps", bufs=1, space="PSUM") as accp,
        ):
            scb_sb = consts.tile([128, 4 * NSEQ], f32)
            nc.sync.dma_start(out=scb_sb, in_=scb[:, :])
            scbB_sb = consts.tile([128, 2 * NST], f32)
            nc.sync.dma_start(out=scbB_sb, in_=scbB[:, :])
            mu_sb = consts.tile([NH, NSEQ * D], f32)
            nc.sync.dma_start(out=mu_sb, in_=mu[:, :])

            # full x broadcast: x_all[:, S*n : S*(n+1)] = xs[n] on all partitions
            x_all = xallp.tile([128, NSEQ * S], f32)
            for n in range(NSEQ):
                nc.sync.dma_start(
                    out=x_all[:, S * n : S * (n + 1)],
                    in_=xs[n : n + 1, :].to_broadcast([128, S]),
                )

            acc_ps = accp.tile([128, NSEQ, D], f32, tag="acc")

            for st in range(NST):
                fstk = fstkp.tile([128, D], f32, tag="fstk")
                # x4: rows [64r:64r+64] = xs[2st+r] (for the B ACTs)
                x4 = x4p.tile([128, S], f32, tag="x4")
                for r in range(2):
                    n = 2 * st + r
                    nc.gpsimd.dma_start(
                        out=x4[64 * r : 64 * (r + 1), :],
                        in_=xs[n : n + 1, :].to_broadcast([64, S]),
                    )

                for r in range(2):
                    n = 2 * st + r
                    lhsF = smallp.tile([NH, GX], f32, tag="lhsF")
                    for tt in range(2):
                        xin = x_all[:, S * n : S * (n + 1)]
                        den = smallp.tile([128, 1], f32, tag="den")
                        et = etp.tile([128, S], f32, tag="et")
                        nc.scalar.activation(
                            out=et,
                            in_=xin,
                            func=Act.Exp,
                            scale=scb_sb[:, 4 * n + 2 * tt : 4 * n + 2 * tt + 1],
                            bias=scb_sb[:, 4 * n + 2 * tt + 1 : 4 * n + 2 * tt + 2],
                            accum_out=den,
                        )
                        rec = smallp.tile([128, 1], f32, tag="rec")
                        nc.vector.reciprocal(rec, den)
                        wf = smallp.tile([128, 1], f32, tag="wf")
                        scr = scrp.tile([128, S], f32, tag="scr")
                        nc.vector.scalar_tensor_tensor(
                            out=scr,
                            in0=et,
                            scalar=rec,
                            in1=xin,
                            op0=Alu.mult,
                            op1=Alu.mult,
                            accum_out=wf,
                        )
                        # wf [128,1] (p=32h+gg) -> lhsF cols [32tt : 32tt+32]
                        nc.sync.dma_start(
                            out=lhsF[:, 32 * tt : 32 * (tt + 1)], in_=wf
                        )
                    f_ps = fpsp.tile([GX, D], f32, tag="fps")
                    nc.tensor.matmul(
                        f_ps,
                        lhsT=lhsF,
                        rhs=mu_sb[:, D * n : D * (n + 1)],
                        start=True,
                        stop=True,
                    )
                    f_sb = smallp.tile([GX, D], f32, tag="fsb")
                    nc.vector.tensor_copy(f_sb, f_ps)
                    nc.sync.dma_start(
                        out=fstk[64 * r : 64 * (r + 1), :], in_=f_sb
                    )

                # B' = min(|x*k1 + (31.5-g)| - 1, 0)  (= -relu(1-|.|))
                babs = babsp.tile([128, S], f32, tag="babs")
                nc.scalar.activation(
                    out=babs,
                    in_=x4,
                    func=Act.Abs,
                    scale=scbB_sb[:, 2 * st : 2 * st + 1],
                    bias=scbB_sb[:, 2 * st + 1 : 2 * st + 2],
                )
                bst = bstp.tile([128, S], f32, tag="bst")
                nc.vector.tensor_scalar(
                    out=bst,
                    in0=babs,
                    scalar1=1.0,
                    scalar2=0.0,
                    op0=Alu.subtract,
                    op1=Alu.min,
                )

                for c in range(NSEQ):
                    nc.tensor.matmul(
                        acc_ps[:, c, :],
                        lhsT=bst[:, 128 * c : 128 * (c + 1)],
                        rhs=fstk,
                        start=(st == 0 and c == 0),
                        stop=(st == NST - 1 and c == NSEQ - 1),
                        skip_group_check=True,
                    )

            out_sb = consts.tile([128, NSEQ, D], f32)
            nc.vector.tensor_copy(out_sb, acc_ps)
            nc.sync.dma_start(
                out=outp.rearrange("(c p) o -> p c o", p=128), in_=out_sb
            )

    if not nc.is_finalized():
        nc.finalize()
    return nc


def _host_inputs(x, embed_w, q_w, k_w, v_w, o_w, merge_w):
    t = np.ascontiguousarray(
        np.asarray(x, np.float64).transpose(0, 3, 1, 2).reshape(B * C, S)
    )
    ev = np.asarray(embed_w, np.float64)[:, 0]
    qv = np.asarray(q_w, np.float64) @ ev
    kv = np.asarray(k_w, np.float64) @ ev
    vv = np.asarray(v_w, np.float64) @ ev
    c_h = np.array(
        [
            qv[DH * h : DH * (h + 1)] @ kv[DH * h : DH * (h + 1)]
            for h in range(NH)
        ]
    ) / np.sqrt(DH)
    U = np.stack(
        [
            np.asarray(o_w, np.float64)[:, DH * h : DH * (h + 1)]
            @ vv[DH * h : DH * (h + 1)]
            for h in range(NH)
        ]
    )  # [4, 64]
    ug = -1.0 + 2.0 * np.arange(GX) / (GX - 1)  # [64]
    # tilt pattern per (partition, tilt-tile): p = 32h + gg, g = 32*tt + gg
    hcu = np.zeros((128, 2), np.float64)
    for tt in range(2):
        hcu[:, tt] = (c_h[:, None] * ug[None, 32 * tt : 32 * (tt + 1)]).reshape(128)
    gcol = np.arange(GX, dtype=np.float64)
    mw = np.asarray(merge_w, np.float64)[0]

    in_maps = []
    for k in range(NCORES):
        rows = t[NSEQ * k : NSEQ * (k + 1)]
        scbv = np.zeros((128, 4 * NSEQ), np.float64)
        scbBv = np.zeros((128, 2 * NST), np.float64)
        muv = np.zeros((NH, NSEQ * D), np.float64)
        for n in range(NSEQ):
            xs_n = rows[n]
            amax = np.abs(xs_n).max()
            for tt in range(2):
                scale = hcu[:, tt] * amax
                bias = -np.maximum(scale * xs_n.max(), scale * xs_n.min())
                scbv[:, 4 * n + 2 * tt] = scale
                scbv[:, 4 * n + 2 * tt + 1] = bias
            merge_n = mw[(NSEQ * k + n) % C]
            muv[:, D * n : D * (n + 1)] = -merge_n * U  # negated (B' sign)
        for st in range(NST):
            for r in range(2):
                n = 2 * st + r
                amax = np.abs(rows[n]).max()
                scbBv[64 * r : 64 * (r + 1), 2 * st] = (GX - 1) / 2.0 / amax
                scbBv[64 * r : 64 * (r + 1), 2 * st + 1] = (GX - 1) / 2.0 - gcol
        in_maps.append(
            dict(
                xs=np.ascontiguousarray(rows.astype(np.float32)),
                scb=np.ascontiguousarray(scbv.astype(np.float32)),
                scbB=np.ascontiguousarray(scbBv.astype(np.float32)),
                mu=np.ascontiguousarray(muv.astype(np.float32)),
            )
        )
    return in_maps


def kernel(x, embed_w, q_w, k_w, v_w, o_w, merge_w):
    from concourse.bass_utils import run_bass_kernel_spmd

    if "nc" not in _CACHE:
        _CACHE["nc"] = _build_nc()
    nc = _CACHE["nc"]
    in_maps = _host_inputs(x, embed_w, q_w, k_w, v_w, o_w, merge_w)
    res = run_bass_kernel_spmd(nc, in_maps, core_ids=list(range(NCORES)))
    out = np.zeros((B, S, D), dtype=np.float32)
    for k in range(NCORES):
        out[k // (NCORES // B)] += res.results[k]["outp"]
    return out.reshape(B, HH, WW, D)
